# revision 14
# baseline (speedup 1.0000x reference)
"""Trainium2 Bass kernel for the 10-class supervised-contrastive loss.

Problem shapes (hardcoded): preds [10, 2048, 128] f32, target [2048] int64,
log_vars [10] f32 -> scalar f32.

The only O(B^2) quantity is Z[c, r] = sum_{j != r} exp(cos(r, j) / T);
everything else (P/R cosine sums via class feature sums, counts, log-prob
assembly) is O(B*D) / O(B*C) and computed on the host in f32.

Sharding (8 cores, SPMD, identical program per core; B=2048 -> 16 strips of
128 rows):
  - slot 0: core c owns class c's full upper trapezoid: strip a computes
    tiles (a, b) for b >= a (136 tiles).
  - slot 1: classes 8 (cores 0-3) and 9 (cores 4-7) are split 4 ways, 34
    tiles each, with an IDENTICAL static program: fed-coordinate tiles
      row 0: cols 0..9 | row 1: cols 1..9 | row 8: cols 8..15 | row 9: 9..15
    Core j feeds the class's features rotated by 2j strips (np.roll by
    256*j rows).  The 4 rotated images of this 34-tile set partition the
    class's 136 unordered strip pairs exactly.

Each unit (strip a, cols [c0,c1)) is split into <=1024-col REGIONS.  The
PSUM budget is one 7-deep pool of [128,1024] cp buffers (7 x 1 bank... 2
banks each? no: [128,1024] f32 = 2 banks; 3 buffers) -- see code: cp pool
holds 3 x [128,1024] (6 banks) + 1 mirror bank + 1 spare; deep enough that
the PE never idles waiting on ACT, so its p-state ramps to 2.4 GHz.

Per region:
  PE   : bf16 matmuls C = G_a^T G[:, r0:r1] (512-col chunks, f32 PSUM).
  ACT  : one Exp(C/T) -> bf16 sc region in SBUF (raw diagonal included,
         it exps to ~e^14.3 which bf16 holds fine).
  GPSIMD: affine_select zeroes the diag window (first 128 cols of each
         unit's first region).
  DVE  : tensor_reduce(X) row-sums sc (excluding the diag window) into a
         per-region f32 column (direct part).
  PE   : zero-padded ones-block stationary matmuls column-sum the full sc
         region (diag tile included -> its masked colsum supplies the
         strip's own off-diagonal terms) into ONE shared mirror PSUM bank:
         row q (slot0) / 4+q (slot1) holds 512-col cell q of the class
         square.  All writes are start=False accumulations onto a memset
         bank; rows below the target row accumulate exact zeros.  The diag
         chunk is emitted as its own matmul so only IT depends on the
         affine_select; csums lag their region by 2 so the in-order PE
         queue always has ready work.

Z[row] = sum of region rowsums + mirror[col of row]; no diag corrections.

Host epilogue: masked mean log-prob from host P/R + analytic counts,
uncertainty-weighted sum.
"""

import ml_dtypes
import numpy as np

import concourse.bacc as bacc
import concourse.bass as bass
import concourse.mybir as mybir
import concourse.tile as tile
from concourse.bass_utils import run_bass_kernel_spmd

NUM_CLASSES = 10
B = 2048
D = 128
T = 0.07
BASE_T = 0.07
N_CORES = 8

f32 = mybir.dt.float32
bf16 = mybir.dt.bfloat16
np_bf16 = ml_dtypes.bfloat16

# Slot-1 static units: fed_row -> (col_start, col_end).
S1 = {0: (0, 1280), 1: (128, 1280), 8: (1024, 2048), 9: (1152, 2048)}

# Unit order: wide first (keeps ACT fed through the DMA ramp), slot-1 units
# late enough that G1 has landed, narrow tail.  (s, idx): s=0 own class
# strip idx; s=1 fed row idx.
ORDER = [
    (0, 0), (0, 8), (0, 1), (0, 9), (0, 2), (0, 10), (0, 3), (0, 11),
    (1, 0), (0, 12), (0, 4), (1, 8), (0, 5), (0, 13), (1, 1), (1, 9),
    (0, 6), (0, 14), (0, 7), (0, 15),
]


def _unit_range(s, idx):
    if s == 0:
        return 128 * idx, 2048
    return S1[idx]


def _regions():
    """Split each unit into <=1024-col regions.
    Returns list of (s, idx, r0, r1, first)."""
    out = []
    for s, idx in ORDER:
        c0, c1 = _unit_range(s, idx)
        r = c0
        first = True
        while r < c1:
            r1 = min(c1, r + 1024)
            out.append((s, idx, r, r1, first))
            r, first = r1, False
    return out


REGIONS = _regions()
N_REG = len(REGIONS)

TRACE = False
LAST_RESULT = None


def _build_nc():
    nc = bacc.Bacc(None, target_bir_lowering=False)

    g_dram = [
        [nc.dram_tensor(f"g{s}c{k}", [128, 512], bf16, kind="ExternalInput")
         for k in range(4)]
        for s in range(2)
    ]
    rs_dram = nc.dram_tensor("rs", [128, N_REG], f32, kind="ExternalOutput")
    mir_dram = nc.dram_tensor("mir", [8, 512], f32, kind="ExternalOutput")

    add = mybir.AluOpType.add
    ne = mybir.AluOpType.not_equal
    EXP = mybir.ActivationFunctionType.Exp

    with tile.TileContext(nc) as tc:
        with (
            tc.tile_pool(name="const", bufs=1) as constp,
            tc.tile_pool(name="gmat", bufs=1) as gmatp,
            tc.tile_pool(name="scp", bufs=6) as scp,
        ):
            # Exp-table preload: a dummy ACTIVATE on an uninitialized scratch
            # tile (no producer, so no wait) pulls the ~1.5us ACT table load
            # into the DMA window instead of the first real call.
            warm = constp.tile([128, 2], bf16, tag="warm")
            nc.scalar.activation(warm[:, 1:2], warm[:, 0:1], EXP)

            # G matrices: one [128, 2048] SBUF tile per slot, filled by four
            # 512-col DMAs so early matmuls only wait for their own chunk.
            # G0 chunks alternate sync/scalar queues (halves the issue
            # serialization for the critical first regions); G1 rides the
            # gpsimd queue (not needed until mid-kernel).
            G = []
            for s in range(2):
                g = gmatp.tile([128, 2048], bf16, tag=f"G{s}", name=f"G{s}")
                G.append(g)
            for k in range(4):
                eng = nc.sync if k % 2 == 0 else nc.scalar
                eng.dma_start(G[0][:, 512 * k : 512 * (k + 1)], g_dram[0][k][:])
            for k in range(4):
                nc.gpsimd.dma_start(G[1][:, 512 * k : 512 * (k + 1)], g_dram[1][k][:])

            # Zero-padded ones block for the mirror column-sum matmuls:
            # opad[:, 7-r : 8] is a [128, r+1] stationary whose rows 0..r-1
            # produce exact-zero accumulands and row r the column sum, so
            # the cell lands at PSUM partition r of the single shared
            # mirror bank while lower rows only accumulate zeros.
            opad = constp.tile([128, 8], bf16, tag="opad")
            nc.vector.memset(opad[:, 0:7], 0.0)
            nc.vector.memset(opad[:, 7:8], 1.0)

            rs_sb = constp.tile([128, N_REG], f32, tag="rs")
            # Unit (0,15)'s region writes no rowsum; zero its column.
            nc.vector.memset(rs_sb[:], 0.0)
            mir_sb = constp.tile([128, 512], f32, tag="mirsb")

            with (
                tc.tile_pool(name="cpp", bufs=1, space="PSUM") as cpp,
                tc.tile_pool(name="mirp", bufs=1, space="PSUM") as mirp,
            ):
                # One big cp tile, manually managed in three 1024-col slots
                # (region n -> slot n%3); one big sc tile in six slots
                # (n%6).  Subtile range tracking provides the WAR deps.
                # Manual slots let one ACT cover both regions of a
                # two-region unit (they sit in adjacent slots: region order
                # follows a pair,pair,single pattern mod 3).
                cpbig = cpp.tile([128, 3072], f32, tag="cp", name="cpbig")
                scbig = constp.tile([128, 6144], bf16, tag="scbig")
                mir = mirp.tile([128, 512], f32, tag="mir", name="mir")
                # All csum matmuls accumulate with start=False, so the
                # mirror cells must begin as zeros.
                nc.vector.memset(mir[0:8, 0:512], 0.0)

                def emit_mains(n, s, idx, r0, r1):
                    W = r1 - r0
                    base = 1024 * (n % 3)
                    lhsT = G[s][:, 128 * idx : 128 * idx + 128]
                    for lo in range(0, W, 512):
                        hi = min(W, lo + 512)
                        nc.tensor.matmul(
                            cpbig[:, base + lo : base + hi],
                            lhsT,
                            G[s][:, r0 + lo : r0 + hi],
                            start=True,
                            stop=True,
                        )

                def emit_post(n, s, idx, r0, r1, first):
                    W = r1 - r0
                    sbase = 1024 * (n % 6)
                    if first:
                        # Zero the diagonal window (the unit's first 128
                        # cols): keep where (partition - col) != 0.  Runs
                        # on the otherwise-idle GpSimd engine.
                        nc.gpsimd.affine_select(
                            scbig[:, sbase : sbase + 128],
                            scbig[:, sbase : sbase + 128],
                            pattern=[[-1, 128]], compare_op=ne, fill=0.0,
                            base=0, channel_multiplier=1,
                        )
                    lo = 128 if first else 0
                    if W > lo:
                        nc.vector.tensor_reduce(
                            rs_sb[:, n : n + 1],
                            scbig[:, sbase + lo : sbase + W],
                            axis=mybir.AxisListType.X, op=add,
                        )

                def emit_csums(n, s, idx, r0, r1, first):
                    # Split [r0,r1) at the 512-cell grid; the diag chunk
                    # (first 128 cols of a first region) goes last and is
                    # the only csum depending on the affine_select.
                    sbase = 1024 * (n % 6)
                    cuts = sorted(
                        {r0, r1}
                        | {b for b in (512, 1024, 1536) if r0 < b < r1}
                        | ({r0 + 128} if first else set())
                    )
                    chunks = list(zip(cuts, cuts[1:]))
                    if first:
                        chunks = chunks[1:] + chunks[:1]
                    for a, b in chunks:
                        q = a // 512
                        assert b <= 512 * (q + 1)
                        row = q if s == 0 else 4 + q
                        nc.tensor.matmul(
                            mir[0 : row + 1, a - 512 * q : b - 512 * q],
                            opad[:, 7 - row : 8],
                            scbig[:, sbase + a - r0 : sbase + b - r0],
                            start=False,
                            stop=True,
                            skip_group_check=True,
                        )

                # Group regions: a two-region unit forms one ACT pair (its
                # regions land in adjacent cp/sc slots); singles stand
                # alone.
                groups = []
                n = 0
                while n < N_REG:
                    if (
                        n + 1 < N_REG
                        and REGIONS[n][4]
                        and not REGIONS[n + 1][4]
                        and REGIONS[n][:2] == REGIONS[n + 1][:2]
                    ):
                        assert n % 3 == 0 and (n + 1) % 3 == 1
                        groups.append([n, n + 1])
                        n += 2
                    else:
                        groups.append([n])
                        n += 1

                lag = []  # groups awaiting csum emission
                done = 0
                for g in groups:
                    for n in g:
                        emit_mains(n, *REGIONS[n][:4])
                    n0 = g[0]
                    Wg = sum(REGIONS[n][3] - REGIONS[n][2] for n in g)
                    nc.scalar.activation(
                        scbig[:, 1024 * (n0 % 6) : 1024 * (n0 % 6) + Wg],
                        cpbig[:, 1024 * (n0 % 3) : 1024 * (n0 % 3) + Wg],
                        EXP,
                        scale=1.0 / T,
                    )
                    for n in g:
                        emit_post(n, *REGIONS[n])
                    lag.append(g)
                    if len(lag) > 2:
                        for n in lag.pop(0):
                            emit_csums(n, *REGIONS[n])
                            done += 1
                    if done == N_REG - 9:
                        # Early rowsums are final; overlap their DMA with
                        # the tail (sync queue is otherwise idle here).
                        nc.sync.dma_start(
                            rs_dram[:, 0 : N_REG - 9], rs_sb[:, 0 : N_REG - 9]
                        )
                        done = -1000
                for g in lag:
                    for n in g:
                        emit_csums(n, *REGIONS[n])

                # Mirror bank -> SBUF (DMA cannot touch PSUM).
                nc.vector.tensor_copy(mir_sb[0:8, 0:512], mir[0:8, 0:512])

            nc.sync.dma_start(rs_dram[:, N_REG - 8 : N_REG], rs_sb[:, N_REG - 8 : N_REG])
            nc.scalar.dma_start(mir_dram[:, :], mir_sb[0:8, 0:512])
    nc.finalize()
    return nc


_NC_CACHE = None


def _get_nc():
    global _NC_CACHE
    if _NC_CACHE is None:
        _NC_CACHE = _build_nc()
    return _NC_CACHE


def kernel(preds, target, log_vars):
    global LAST_RESULT
    preds = np.asarray(preds, dtype=np.float32)
    target = np.asarray(target)
    log_vars = np.asarray(log_vars, dtype=np.float32)

    onehot = (target[None, :] == np.arange(NUM_CLASSES, dtype=target.dtype)[:, None])
    onehot = onehot.astype(np.float32)  # [10, B]
    npos = onehot.sum(axis=1).astype(np.float64)  # [10]

    # Host prep: row-normalize (f32 stats), cast bf16, d-major layout.
    norms = np.sqrt((preds.astype(np.float32) ** 2).sum(axis=2, dtype=np.float32))
    ghat32 = preds / norms[:, :, None]  # [10, B, D] f32
    ghat = ghat32.astype(np_bf16)

    # Host P/R: per-row cosine sums against positives / all rows (f32).
    u_all = ghat32.sum(axis=1)  # [10, D]
    u_pos = np.einsum("cbd,cb->cd", ghat32, onehot)  # [10, D]
    P = np.einsum("cbd,cd->cb", ghat32, u_pos)  # [10, B]
    R = np.einsum("cbd,cd->cb", ghat32, u_all)  # [10, B]

    in_maps = []
    for c in range(N_CORES):
        cls1 = 8 + c // 4
        off = 256 * (c % 4)  # rotation: fed strip f = actual strip f + 2j
        im = {}
        for s, (cls, o) in enumerate([(c, 0), (cls1, off)]):
            gh = np.roll(ghat[cls], -o, axis=0) if o else ghat[cls]
            gt = np.ascontiguousarray(gh.T)  # [128, 2048] [d, b]
            for k in range(4):
                im[f"g{s}c{k}"] = np.ascontiguousarray(gt[:, 512 * k : 512 * (k + 1)])
        in_maps.append(im)

    nc = _get_nc()
    res = run_bass_kernel_spmd(nc, in_maps, list(range(N_CORES)), trace=TRACE)
    LAST_RESULT = res

    # Region -> rs column map: unit -> list of region indices.
    unit_regs = {}
    for n, (s, idx, r0, r1, first) in enumerate(REGIONS):
        unit_regs.setdefault((s, idx), []).append(n)

    # Assemble Z (sum over j != i of exp(cos_ij / T)) from partials.
    Z = np.zeros((NUM_CLASSES, B), dtype=np.float64)
    for c in range(N_CORES):
        rs = np.asarray(res.results[c]["rs"], dtype=np.float64)    # [128, N_REG]
        mir = np.asarray(res.results[c]["mir"], dtype=np.float64)  # [8, 512]
        for b in range(16):
            g0 = 128 * b
            z = mir[g0 // 512, g0 % 512 : g0 % 512 + 128].copy()
            for n in unit_regs[(0, b)]:
                z += rs[:, n]
            Z[c, g0 : g0 + 128] = z
    for cls in (8, 9):
        cores = range(0, 4) if cls == 8 else range(4, 8)
        for t in range(16):
            acc = np.zeros(128, dtype=np.float64)
            for c in cores:
                j = c % 4
                f = (t - 2 * j) % 16
                g0 = 128 * f
                mir = np.asarray(res.results[c]["mir"], dtype=np.float64)
                acc += mir[4 + g0 // 512, g0 % 512 : g0 % 512 + 128]
                if f in S1:
                    rs = np.asarray(res.results[c]["rs"], dtype=np.float64)
                    for n in unit_regs[(1, f)]:
                        acc += rs[:, n]
            Z[cls, 128 * t : 128 * t + 128] = acc

    lab = onehot.astype(np.float64)
    masked_cos = lab * P.astype(np.float64) + (1.0 - lab) * (R - P).astype(np.float64)
    masked_logits_sum = (masked_cos - 1.0) / T
    cnt = lab * npos[:, None] + (1.0 - lab) * (B - npos[:, None]) - 1.0
    mlpp = masked_logits_sum / cnt - np.log(Z)
    losses = -(T / BASE_T) * mlpp.mean(axis=1)  # [10]
    lv = log_vars.astype(np.float64)
    final = np.sum(np.exp(-lv) * losses + lv)
    return np.float32(final)


# revision 16
# speedup vs baseline: 1.3570x; 1.3570x over previous
"""Trainium2 Bass kernel for the 10-class supervised-contrastive loss.

Problem shapes (hardcoded): preds [10, 2048, 128] f32, target [2048] int64,
log_vars [10] f32 -> scalar f32.

The only O(B^2) quantity is Z[c, r] = sum_{j != r} exp(cos(r, j) / T);
everything else (P/R cosine sums via class feature sums, counts, log-prob
assembly) is O(B*D) / O(B*C) and computed on the host in f32.

Sharding (8 cores, SPMD, identical program per core; B=2048 -> 16 strips of
128 rows):
  - slot 0: core c owns class c's full upper trapezoid: strip a computes
    tiles (a, b) for b >= a (136 tiles).
  - slot 1: classes 8 (cores 0-3) and 9 (cores 4-7) are split 4 ways, 34
    tiles each, with an IDENTICAL static program: fed-coordinate tiles
      row 0: cols 0..9 | row 1: cols 1..9 | row 8: cols 8..15 | row 9: 9..15
    Core j feeds the class's features rotated by 2j strips (np.roll by
    256*j rows).  The 4 rotated images of this 34-tile set partition the
    class's 136 unordered strip pairs exactly.

Each unit (strip a, cols [c0,c1)) is split into <=1024-col REGIONS.  The
PSUM budget is one 7-deep pool of [128,1024] cp buffers (7 x 1 bank... 2
banks each? no: [128,1024] f32 = 2 banks; 3 buffers) -- see code: cp pool
holds 3 x [128,1024] (6 banks) + 1 mirror bank + 1 spare; deep enough that
the PE never idles waiting on ACT, so its p-state ramps to 2.4 GHz.

Per region:
  PE   : bf16 matmuls C = G_a^T G[:, r0:r1] (512-col chunks, f32 PSUM).
  ACT  : one Exp(C/T) -> bf16 sc region in SBUF (raw diagonal included,
         it exps to ~e^14.3 which bf16 holds fine).
  GPSIMD: affine_select zeroes the diag window (first 128 cols of each
         unit's first region).
  DVE  : tensor_reduce(X) row-sums sc (excluding the diag window) into a
         per-region f32 column (direct part).
  PE   : zero-padded ones-block stationary matmuls column-sum the full sc
         region (diag tile included -> its masked colsum supplies the
         strip's own off-diagonal terms) into ONE shared mirror PSUM bank:
         row q (slot0) / 4+q (slot1) holds 512-col cell q of the class
         square.  All writes are start=False accumulations onto a memset
         bank; rows below the target row accumulate exact zeros.  The diag
         chunk is emitted as its own matmul so only IT depends on the
         affine_select; csums lag their region by 2 so the in-order PE
         queue always has ready work.

Z[row] = sum of region rowsums + mirror[col of row]; no diag corrections.

Host epilogue: masked mean log-prob from host P/R + analytic counts,
uncertainty-weighted sum.
"""

import ml_dtypes
import numpy as np

import concourse.bacc as bacc
import concourse.bass as bass
import concourse.mybir as mybir
import concourse.tile as tile
from concourse.bass_utils import run_bass_kernel_spmd

NUM_CLASSES = 10
B = 2048
D = 128
T = 0.07
BASE_T = 0.07
N_CORES = 8

f32 = mybir.dt.float32
bf16 = mybir.dt.bfloat16
np_bf16 = ml_dtypes.bfloat16

# Slot-1 static units: fed_row -> (col_start, col_end).
S1 = {0: (0, 1280), 1: (128, 1280), 8: (1024, 2048), 9: (1152, 2048)}

# Unit order: wide first (keeps ACT fed through the DMA ramp), slot-1 units
# late enough that G1 has landed, narrow tail.  (s, idx): s=0 own class
# strip idx; s=1 fed row idx.
ORDER = [
    (0, 0), (0, 8), (0, 1), (0, 9), (0, 2), (0, 10), (0, 3), (0, 11),
    (1, 0), (0, 12), (0, 4), (1, 8), (0, 5), (0, 13), (1, 1), (1, 9),
    (0, 6), (0, 14), (0, 7), (0, 15),
]


def _unit_range(s, idx):
    if s == 0:
        return 128 * idx, 2048
    return S1[idx]


def _regions():
    """Split each unit into <=1024-col regions.
    Returns list of (s, idx, r0, r1, first)."""
    out = []
    for s, idx in ORDER:
        c0, c1 = _unit_range(s, idx)
        r = c0
        first = True
        while r < c1:
            r1 = min(c1, r + 1024)
            out.append((s, idx, r, r1, first))
            r, first = r1, False
    return out


REGIONS = _regions()
N_REG = len(REGIONS)

TRACE = False
LAST_RESULT = None


def _build_nc():
    nc = bacc.Bacc(None, target_bir_lowering=False)

    g_dram = [
        [nc.dram_tensor(f"g{s}c{k}", [128, 512], bf16, kind="ExternalInput")
         for k in range(4)]
        for s in range(2)
    ]
    rs_dram = nc.dram_tensor("rs", [128, N_REG], f32, kind="ExternalOutput")
    mir_dram = nc.dram_tensor("mir", [8, 512], f32, kind="ExternalOutput")

    add = mybir.AluOpType.add
    ne = mybir.AluOpType.not_equal
    EXP = mybir.ActivationFunctionType.Exp

    with tile.TileContext(nc) as tc:
        with (
            tc.tile_pool(name="const", bufs=1) as constp,
            tc.tile_pool(name="gmat", bufs=1) as gmatp,
            tc.tile_pool(name="scp", bufs=6) as scp,
        ):
            # Exp-table preload: a dummy ACTIVATE on an uninitialized scratch
            # tile (no producer, so no wait) pulls the ~1.5us ACT table load
            # into the DMA window instead of the first real call.
            warm = constp.tile([128, 2], bf16, tag="warm")
            nc.scalar.activation(warm[:, 1:2], warm[:, 0:1], EXP)

            # G matrices: one [128, 2048] SBUF tile per slot, filled by four
            # 512-col DMAs so early matmuls only wait for their own chunk.
            # G0 chunks alternate sync/scalar queues (halves the issue
            # serialization for the critical first regions); G1 rides the
            # gpsimd queue (not needed until mid-kernel).
            G = []
            for s in range(2):
                g = gmatp.tile([128, 2048], bf16, tag=f"G{s}", name=f"G{s}")
                G.append(g)
            for k in range(4):
                eng = nc.sync if k % 2 == 0 else nc.scalar
                eng.dma_start(G[0][:, 512 * k : 512 * (k + 1)], g_dram[0][k][:])
            for k in range(4):
                nc.gpsimd.dma_start(G[1][:, 512 * k : 512 * (k + 1)], g_dram[1][k][:])

            # Zero-padded ones block for the mirror column-sum matmuls:
            # opad[:, 7-r : 8] is a [128, r+1] stationary whose rows 0..r-1
            # produce exact-zero accumulands and row r the column sum, so
            # the cell lands at PSUM partition r of the single shared
            # mirror bank while lower rows only accumulate zeros.
            opad = constp.tile([128, 8], bf16, tag="opad")
            nc.vector.memset(opad[:, 0:7], 0.0)
            nc.vector.memset(opad[:, 7:8], 1.0)

            rs_sb = constp.tile([128, N_REG], f32, tag="rs")
            # Unit (0,15)'s region writes no rowsum; zero its column.
            nc.vector.memset(rs_sb[:], 0.0)
            mir_sb = constp.tile([128, 512], f32, tag="mirsb")

            with (
                tc.tile_pool(name="cpp", bufs=3, space="PSUM") as cpp,
                tc.tile_pool(name="mirp", bufs=1, space="PSUM") as mirp,
            ):
                mir = mirp.tile([128, 512], f32, tag="mir", name="mir")
                # All csum matmuls accumulate with start=False, so the
                # mirror cells must begin as zeros.
                nc.vector.memset(mir[0:8, 0:512], 0.0)

                def emit_mains(s, idx, r0, r1):
                    W = r1 - r0
                    cp = cpp.tile([128, 1024], f32, tag="cp", name=f"cp_{s}_{idx}_{r0}")
                    lhsT = G[s][:, 128 * idx : 128 * idx + 128]
                    for lo in range(0, W, 512):
                        hi = min(W, lo + 512)
                        nc.tensor.matmul(
                            cp[:, lo:hi],
                            lhsT,
                            G[s][:, r0 + lo : r0 + hi],
                            start=True,
                            stop=True,
                        )
                    return cp

                def emit_act(n, s, idx, r0, r1, first, cp):
                    W = r1 - r0
                    sc = scp.tile([128, 1024], bf16, tag="sc", name=f"sc{n}")
                    nc.scalar.activation(sc[:, 0:W], cp[:, 0:W], EXP, scale=1.0 / T)
                    if first:
                        # Zero the diagonal window (the unit's first 128
                        # cols): keep where (partition - col) != 0.  Runs on
                        # the otherwise-idle GpSimd engine.
                        nc.gpsimd.affine_select(
                            sc[:, 0:128], sc[:, 0:128],
                            pattern=[[-1, 128]], compare_op=ne, fill=0.0,
                            base=0, channel_multiplier=1,
                        )
                    lo = 128 if first else 0
                    if W > lo:
                        nc.vector.tensor_reduce(
                            rs_sb[:, n : n + 1], sc[:, lo:W],
                            axis=mybir.AxisListType.X, op=add,
                        )
                    return sc

                def emit_csums(s, idx, r0, r1, first, sc):
                    # Split [r0,r1) at the 512-cell grid; the diag chunk
                    # (first 128 cols of a first region) goes last and is
                    # the only csum depending on the affine_select.
                    cuts = sorted(
                        {r0, r1}
                        | {b for b in (512, 1024, 1536) if r0 < b < r1}
                        | ({r0 + 128} if first else set())
                    )
                    chunks = list(zip(cuts, cuts[1:]))
                    if first:
                        chunks = chunks[1:] + chunks[:1]
                    for a, b in chunks:
                        q = a // 512
                        assert b <= 512 * (q + 1)
                        row = q if s == 0 else 4 + q
                        nc.tensor.matmul(
                            mir[0 : row + 1, a - 512 * q : b - 512 * q],
                            opad[:, 7 - row : 8],
                            sc[:, a - r0 : b - r0],
                            start=False,
                            stop=True,
                            skip_group_check=True,
                        )

                lag = []  # regions awaiting csum emission (depth 2)
                for n, (s, idx, r0, r1, first) in enumerate(REGIONS):
                    cp = emit_mains(s, idx, r0, r1)
                    sc = emit_act(n, s, idx, r0, r1, first, cp)
                    lag.append((s, idx, r0, r1, first, sc))
                    if len(lag) > 2:
                        emit_csums(*lag.pop(0))
                    if n == N_REG - 7:
                        # Early rowsums are final; overlap their DMA with
                        # the tail (sync queue is idle here).
                        nc.sync.dma_start(
                            rs_dram[:, 0 : N_REG - 8], rs_sb[:, 0 : N_REG - 8]
                        )
                for item in lag:
                    emit_csums(*item)

                # Mirror bank -> SBUF (DMA cannot touch PSUM).
                nc.vector.tensor_copy(mir_sb[0:8, 0:512], mir[0:8, 0:512])

            # Probe: does DVE tensor_tensor hit the 2x 16-bit mode?  Two
            # [128,512] all-bf16 adds on scratch data, off the critical
            # path (after the last output DMA); read their duration from
            # the trace.  TODO remove.
            probe = constp.tile([128, 1536], bf16, tag="probe")
            nc.vector.tensor_tensor(
                out=probe[:, 1024:1536], in0=probe[:, 0:512],
                in1=probe[:, 512:1024], op=add,
            )

            nc.sync.dma_start(rs_dram[:, N_REG - 8 : N_REG], rs_sb[:, N_REG - 8 : N_REG])
            nc.scalar.dma_start(mir_dram[:, :], mir_sb[0:8, 0:512])
    nc.finalize()
    return nc


_NC_CACHE = None


def _get_nc():
    global _NC_CACHE
    if _NC_CACHE is None:
        _NC_CACHE = _build_nc()
    return _NC_CACHE


def kernel(preds, target, log_vars):
    global LAST_RESULT
    preds = np.asarray(preds, dtype=np.float32)
    target = np.asarray(target)
    log_vars = np.asarray(log_vars, dtype=np.float32)

    onehot = (target[None, :] == np.arange(NUM_CLASSES, dtype=target.dtype)[:, None])
    onehot = onehot.astype(np.float32)  # [10, B]
    npos = onehot.sum(axis=1).astype(np.float64)  # [10]

    # Host prep: row-normalize (f32 stats), cast bf16, d-major layout.
    norms = np.sqrt((preds.astype(np.float32) ** 2).sum(axis=2, dtype=np.float32))
    ghat32 = preds / norms[:, :, None]  # [10, B, D] f32
    ghat = ghat32.astype(np_bf16)

    # Host P/R: per-row cosine sums against positives / all rows (f32).
    u_all = ghat32.sum(axis=1)  # [10, D]
    u_pos = np.einsum("cbd,cb->cd", ghat32, onehot)  # [10, D]
    P = np.einsum("cbd,cd->cb", ghat32, u_pos)  # [10, B]
    R = np.einsum("cbd,cd->cb", ghat32, u_all)  # [10, B]

    in_maps = []
    for c in range(N_CORES):
        cls1 = 8 + c // 4
        off = 256 * (c % 4)  # rotation: fed strip f = actual strip f + 2j
        im = {}
        for s, (cls, o) in enumerate([(c, 0), (cls1, off)]):
            gh = np.roll(ghat[cls], -o, axis=0) if o else ghat[cls]
            gt = np.ascontiguousarray(gh.T)  # [128, 2048] [d, b]
            for k in range(4):
                im[f"g{s}c{k}"] = np.ascontiguousarray(gt[:, 512 * k : 512 * (k + 1)])
        in_maps.append(im)

    nc = _get_nc()
    res = run_bass_kernel_spmd(nc, in_maps, list(range(N_CORES)), trace=TRACE)
    LAST_RESULT = res

    # Region -> rs column map: unit -> list of region indices.
    unit_regs = {}
    for n, (s, idx, r0, r1, first) in enumerate(REGIONS):
        unit_regs.setdefault((s, idx), []).append(n)

    # Assemble Z (sum over j != i of exp(cos_ij / T)) from partials.
    Z = np.zeros((NUM_CLASSES, B), dtype=np.float64)
    for c in range(N_CORES):
        rs = np.asarray(res.results[c]["rs"], dtype=np.float64)    # [128, N_REG]
        mir = np.asarray(res.results[c]["mir"], dtype=np.float64)  # [8, 512]
        for b in range(16):
            g0 = 128 * b
            z = mir[g0 // 512, g0 % 512 : g0 % 512 + 128].copy()
            for n in unit_regs[(0, b)]:
                z += rs[:, n]
            Z[c, g0 : g0 + 128] = z
    for cls in (8, 9):
        cores = range(0, 4) if cls == 8 else range(4, 8)
        for t in range(16):
            acc = np.zeros(128, dtype=np.float64)
            for c in cores:
                j = c % 4
                f = (t - 2 * j) % 16
                g0 = 128 * f
                mir = np.asarray(res.results[c]["mir"], dtype=np.float64)
                acc += mir[4 + g0 // 512, g0 % 512 : g0 % 512 + 128]
                if f in S1:
                    rs = np.asarray(res.results[c]["rs"], dtype=np.float64)
                    for n in unit_regs[(1, f)]:
                        acc += rs[:, n]
            Z[cls, 128 * t : 128 * t + 128] = acc

    lab = onehot.astype(np.float64)
    masked_cos = lab * P.astype(np.float64) + (1.0 - lab) * (R - P).astype(np.float64)
    masked_logits_sum = (masked_cos - 1.0) / T
    cnt = lab * npos[:, None] + (1.0 - lab) * (B - npos[:, None]) - 1.0
    mlpp = masked_logits_sum / cnt - np.log(Z)
    losses = -(T / BASE_T) * mlpp.mean(axis=1)  # [10]
    lv = log_vars.astype(np.float64)
    final = np.sum(np.exp(-lv) * losses + lv)
    return np.float32(final)
